# revision 8
# baseline (speedup 1.0000x reference)
"""Trainium2 Bass kernel for the contextual channel-attention transformer block.

Contract: kernel(**inputs) takes the FULL unsharded inputs
(x: (8,512,64,64) f32, Wq/Wk/Wv: (512,512) f32, gamma: (1,) f32) and
returns the FULL (8,512,64,64) f32 output.  Internally the batch is
data-parallel across 8 NeuronCores (one batch element per core).

Per-core algorithm:
  Gx   = X @ X.T                fp8 DoubleRow matmuls (2x rate)
  M3q  = Gx @ Wq.T, M3k = Gx @ Wk.T    bf16
  G^T  = Wk @ M3q  = (Q @ K.T).T       bf16
  cos -> col-max -> temperature -> softmax on G^T[d, c]
  A^T  = Wv.T @ Msm^T                  bf16, then scaled by the per-row
         L1 norm * gamma * 2^k and cast to fp8
  out  = A^T.T @ X              fp8 DoubleRow
  y'   = psum + 2^k*x(bf16)     one vector/gpsimd add, y' stored bf16
Host divides by 2^k when casting the output back to fp32.
"""

import os
import sys

for _p in ("/opt/trn_rl_repo", "/root/.axon_site/_ro/trn_rl_repo"):
    if os.path.isdir(_p) and _p not in sys.path:
        sys.path.insert(0, _p)

import math

import ml_dtypes
import numpy as np

import concourse.bass as bass
import concourse.tile as tile
from concourse import bacc, bass_utils, mybir

# Problem constants (hardcoded; kernel.py must be self-contained).
B, C, HH, WW = 8, 512, 64, 64
N = HH * WW          # 4096 spatial positions
G = C // 128         # 4 channel groups of 128
N1 = N // 128        # 32 Gram chunks (128 spatial each)
NJ = N // 512        # 8 output chunks (512 spatial each)
XT_PIECES = (2, 2, 4, 8, 16)   # progressive xt chunks (units of 128 spatial)
EPS = 1e-6
INV_H = 4.0          # 1 / 0.25 temperature
FP32 = mybir.dt.float32
BF16 = mybir.dt.bfloat16
FP8 = mybir.dt.float8e4
DR = mybir.MatmulPerfMode.DoubleRow

_CACHE = {}


def _build_nc():
    nc = bacc.Bacc("TRN2", target_bir_lowering=False)

    xt_d = nc.dram_tensor("xt", [N, C], FP8, kind="ExternalInput")    # x^T
    xh_d = nc.dram_tensor("xh", [C, N], FP8, kind="ExternalInput")
    xr_d = nc.dram_tensor("xr", [C, N], BF16, kind="ExternalInput")   # 2^k x
    wqt_d = nc.dram_tensor("wqt", [C, C], BF16, kind="ExternalInput")  # Wq^T
    wkt_d = nc.dram_tensor("wkt", [C, C], BF16, kind="ExternalInput")  # Wk^T
    wvo_d = nc.dram_tensor("wvo", [C, C], BF16, kind="ExternalInput")  # Wv
    gcol_d = nc.dram_tensor("gamma_col", [128, 1], FP32, kind="ExternalInput")
    ocol_d = nc.dram_tensor("ones_col", [128, 1], BF16, kind="ExternalInput")
    orow_d = nc.dram_tensor("ones_row", [1, C], BF16, kind="ExternalInput")
    y_d = nc.dram_tensor("y", [C, N], BF16, kind="ExternalOutput")

    xt_v = xt_d.ap().rearrange("(i p) c -> p i c", p=128)    # [128, N1, C]
    xh_v = xh_d.ap().rearrange("(g p) n -> p g n", p=128)    # [128, G, N]
    xr_v = xr_d.ap().rearrange("(g p) n -> p g n", p=128)
    wq_v = wqt_d.ap().rearrange("(g p) o -> p g o", p=128)   # [128, G, C]
    wk_v = wkt_d.ap().rearrange("(g p) o -> p g o", p=128)
    wv_v = wvo_d.ap().rearrange("(g p) o -> p g o", p=128)
    y_v = y_d.ap().rearrange("(g p) n -> p g n", p=128)

    MUL = mybir.AluOpType.mult
    ADD = mybir.AluOpType.add
    MIN = mybir.AluOpType.min
    AX = mybir.AxisListType.X
    Exp = mybir.ActivationFunctionType.Exp
    Ln = mybir.ActivationFunctionType.Ln
    Copy = mybir.ActivationFunctionType.Copy

    with tile.TileContext(nc) as tc:
        with (
            tc.tile_pool(name="consts", bufs=1) as cpool,
            tc.tile_pool(name="weights", bufs=1) as wpool,
            tc.tile_pool(name="xt", bufs=1) as xtpool,
            tc.tile_pool(name="xbig", bufs=1) as xbpool,
            tc.tile_pool(name="gram", bufs=1) as gpool,
            tc.tile_pool(name="small", bufs=2) as spool,
            tc.tile_pool(name="mid", bufs=3) as mpool,
            tc.tile_pool(name="msm", bufs=1) as msmpool,
            tc.tile_pool(name="outs", bufs=4) as opool,
        ):
            # ---- input DMAs (xt first: Gx depends only on it) ------------
            xt_t = []
            off = 0
            for pi, sz in enumerate(XT_PIECES):
                t = xtpool.tile([128, sz, C], FP8, tag=f"xt{pi}", bufs=1,
                                name=f"xt{pi}")
                nc.sync.dma_start(t[:], xt_v[:, off:off + sz, :])
                xt_t.append((off, sz, t))
                off += sz

            ones_col = cpool.tile([128, 1], BF16, tag="ones_col")
            nc.scalar.dma_start(ones_col[:], ocol_d.ap())
            ones_row = cpool.tile([1, C], BF16, tag="ones_row")
            nc.scalar.dma_start(ones_row[:], orow_d.ap())
            gamma_col = cpool.tile([128, 1], FP32, tag="gamma_col")
            nc.scalar.dma_start(gamma_col[:], gcol_d.ap())

            wq = wpool.tile([128, G, C], BF16, tag="wq")
            wk = wpool.tile([128, G, C], BF16, tag="wk")
            wv = wpool.tile([128, G, C], BF16, tag="wv")
            nc.scalar.dma_start(wq[:], wq_v)
            nc.scalar.dma_start(wk[:], wk_v)
            nc.scalar.dma_start(wv[:], wv_v)

            # whole-tensor loads; consumed only in phase 2
            xh_t = xbpool.tile([128, G, N], FP8, tag="xh", name="xh")
            nc.gpsimd.dma_start(xh_t[:], xh_v)
            xr_t = xbpool.tile([128, G, N], BF16, tag="xr", name="xr")
            nc.gpsimd.dma_start(xr_t[:], xr_v)

            # ---- Gx = X X^T  (fp8 DoubleRow, accumulated over 16 pairs) --
            gx_sb = gpool.tile([128, G, C], BF16, tag="gx_sb")
            with tc.tile_pool(name="psGx", bufs=1, space="PSUM") as psGx:
                gx_ps = [psGx.tile([128, C], FP32, tag="gx", bufs=G,
                                   name=f"gx{cg}") for cg in range(G)]
                first = True
                for off, sz, t in xt_t:
                    for lp in range(sz // 2):
                        last = (off + 2 * lp + 2 == N1)
                        for cg in range(G):
                            nc.tensor.matmul(
                                gx_ps[cg][:],
                                t[:, 2 * lp:2 * lp + 2,
                                  cg * 128:(cg + 1) * 128],
                                t[:, 2 * lp:2 * lp + 2, :],
                                start=first, stop=last, perf_mode=DR)
                        first = False
                for cg in range(G):
                    eng = nc.scalar.copy if cg % 2 else nc.vector.tensor_copy
                    eng(gx_sb[:, cg, :], gx_ps[cg][:])

            # ---- M3q = Gx Wq^T, M3k = Gx Wk^T (bf16) ---------------------
            m3q = gpool.tile([128, G, C], BF16, tag="m3q")
            m3k = gpool.tile([128, G, C], BF16, tag="m3k")
            with tc.tile_pool(name="psM3", bufs=1, space="PSUM") as psM3:
                for cg in range(G):
                    q_ps = psM3.tile([128, C], FP32, tag="m3q", bufs=G,
                                     name=f"m3q{cg}")
                    k_ps = psM3.tile([128, C], FP32, tag="m3k", bufs=G,
                                     name=f"m3k{cg}")
                    for g in range(G):
                        lhs = gx_sb[:, g, cg * 128:(cg + 1) * 128]
                        nc.tensor.matmul(q_ps[:], lhs, wq[:, g, :],
                                         start=(g == 0), stop=(g == G - 1))
                        nc.tensor.matmul(k_ps[:], lhs, wk[:, g, :],
                                         start=(g == 0), stop=(g == G - 1))
                    nc.scalar.copy(m3q[:, cg, :], q_ps[:])
                    nc.vector.tensor_copy(m3k[:, cg, :], k_ps[:])

            msm = msmpool.tile([128, G, C], BF16, tag="msm")
            at_f8 = gpool.tile([128, G, C], FP8, tag="at_f8")
            with tc.tile_pool(name="psN", bufs=1, space="PSUM") as psN:
                # ---- norms: |Q_c|^2 row, |K_d|^2 columns -----------------
                sqq = psN.tile([1, C], FP32, tag="sqq", name="sqq")
                sqk_ps = [psN.tile([128, 1], FP32, tag="sqk", bufs=G,
                                   name=f"sqk{d}") for d in range(G)]
                for g in range(G):
                    tq = mpool.tile([128, C], BF16, tag="tq")
                    nc.vector.tensor_tensor(tq[:], wq[:, g, :], m3q[:, g, :],
                                            op=MUL)
                    nc.tensor.matmul(sqq[:], ones_col[:], tq[:],
                                     start=(g == 0), stop=(g == G - 1))
                    tk = mpool.tile([128, C], BF16, tag="tk")
                    nc.gpsimd.tensor_tensor(tk[:], wk[:, g, :],
                                            m3k[:, g, :], op=MUL)
                    for dg in range(G):
                        nc.tensor.matmul(sqk_ps[dg][:],
                                         tk[:, dg * 128:(dg + 1) * 128],
                                         ones_col[:],
                                         start=(g == 0), stop=(g == G - 1))

                # rq row (bf16, for broadcast matmul); rk columns (fp32)
                # 1/sqrt(s) = exp(-0.5*ln(s))
                ln_q = spool.tile([1, C], FP32, tag="ln_q")
                nc.scalar.activation(ln_q[:], sqq[:], Ln)
                ln_ks = []
                for dg in range(G):
                    ln_k = spool.tile([128, 1], FP32, tag="ln_k", bufs=G,
                                      name=f"ln_k{dg}")
                    nc.scalar.activation(ln_k[:], sqk_ps[dg][:], Ln)
                    ln_ks.append(ln_k)
                rq_bf = spool.tile([1, C], BF16, tag="rq_bf")
                nc.scalar.activation(rq_bf[:], ln_q[:], Exp, scale=-0.5)
                rk_cols = []
                for dg in range(G):
                    rk = spool.tile([128, 1], FP32, tag="rk", bufs=G,
                                    name=f"rk{dg}")
                    nc.scalar.activation(rk[:], ln_ks[dg][:], Exp, scale=-0.5)
                    rk_cols.append(rk)

                bq_ps = psN.tile([128, C], FP32, tag="bq_ps", name="bq_ps")
                nc.tensor.matmul(bq_ps[:], ones_row[:, 0:128], rq_bf[:],
                                 start=True, stop=True)
                bq = mpool.tile([128, C], FP32, tag="bq", bufs=1)
                nc.scalar.copy(bq[:], bq_ps[:])

            with tc.tile_pool(name="psB", bufs=1, space="PSUM") as psB:
                # ---- G^T per d-group + softmax + A^T ---------------------
                at_ps = [psB.tile([128, C], FP32, tag="at", bufs=G,
                                  name=f"at{eg}") for eg in range(G)]
                for dg in range(G):
                    big = nc.vector if dg % 2 == 0 else nc.gpsimd
                    g_ps = psB.tile([128, C], FP32, tag="g_ps", bufs=2,
                                    name=f"g_ps{dg}")
                    for g in range(G):
                        nc.tensor.matmul(g_ps[:],
                                         wk[:, g, dg * 128:(dg + 1) * 128],
                                         m3q[:, g, :],
                                         start=(g == 0), stop=(g == G - 1))
                    # cos = G^T * rq_c * rk_d  (vector reads PSUM; gpsimd
                    # cannot, so it gets the SBUF-only followups)
                    t1 = mpool.tile([128, C], FP32, tag="t1")
                    nc.vector.tensor_tensor(t1[:], g_ps[:], bq[:], op=MUL)
                    cosd = mpool.tile([128, C], FP32, tag="cosd")
                    big.tensor_scalar(cosd[:], t1[:], rk_cols[dg][:],
                                      None, op0=MUL)
                    mn = spool.tile([128, 1], FP32, tag="mn")
                    nc.vector.tensor_reduce(mn[:], cosd[:], axis=AX, op=MIN)
                    den = spool.tile([128, 1], FP32, tag="den")
                    nc.vector.tensor_scalar(den[:], mn[:], -1.0, 1.0 + EPS,
                                            op0=MUL, op1=ADD)
                    r = spool.tile([128, 1], FP32, tag="r")
                    nc.vector.reciprocal(r[:], den[:])
                    sv = spool.tile([128, 1], FP32, tag="sv")
                    nc.vector.tensor_scalar(sv[:], r[:], INV_H, 0.0,
                                            op0=MUL, op1=ADD)
                    bv = spool.tile([128, 1], FP32, tag="bv")
                    nc.vector.tensor_scalar(bv[:], r[:], -INV_H, 1.0,
                                            op0=MUL, op1=ADD)
                    e = mpool.tile([128, C], BF16, tag="e")
                    se = spool.tile([128, 1], FP32, tag="se")
                    nc.scalar.activation(e[:], cosd[:], Exp,
                                         bias=bv[:], scale=sv[:],
                                         accum_out=se[:])
                    rd = spool.tile([128, 1], FP32, tag="rd")
                    nc.vector.reciprocal(rd[:], se[:])
                    big.tensor_scalar(msm[:, dg, :], e[:], rd[:], None,
                                      op0=MUL)
                    # A^T accumulation over d
                    for eg in range(G):
                        nc.tensor.matmul(at_ps[eg][:],
                                         wv[:, dg, eg * 128:(eg + 1) * 128],
                                         msm[:, dg, :],
                                         start=(dg == 0), stop=(dg == G - 1))

                # ---- row-L1 sums as a row -> f = 2^k*gamma/rowsum --------
                # s_row[0, c] = sum_d Msm[c, d] via ones_col matmuls
                s_row = psB.tile([1, C], FP32, tag="s_row", name="s_row")
                for dg in range(G):
                    nc.tensor.matmul(s_row[:], ones_col[:], msm[:, dg, :],
                                     start=(dg == 0), stop=(dg == G - 1))
                speps = spool.tile([1, C], FP32, tag="speps", bufs=1)
                nc.vector.tensor_scalar(speps[:], s_row[:], EPS, None,
                                        op0=ADD)
                rs_row = spool.tile([1, C], FP32, tag="rs_row", bufs=1)
                nc.vector.reciprocal(rs_row[:], speps[:])
                f_row = spool.tile([1, C], BF16, tag="f_row", bufs=1)
                nc.vector.tensor_scalar(f_row[:], rs_row[:],
                                        gamma_col[0:1, :], None, op0=MUL)
                fbc_ps = psB.tile([128, C], FP32, tag="fbc_ps", name="fbc_ps")
                nc.tensor.matmul(fbc_ps[:], ones_row[:, 0:128], f_row[:],
                                 start=True, stop=True)
                fbc = mpool.tile([128, C], FP32, tag="fbc", bufs=1)
                nc.scalar.copy(fbc[:], fbc_ps[:])

                # ---- A^T * f -> fp8 (PSUM read -> vector only) -----------
                for eg in range(G):
                    nc.vector.tensor_tensor(at_f8[:, eg, :], at_ps[eg][:],
                                            fbc[:], op=MUL)

            # ---- phase 2: psum = A^T.T X * 2^k*gamma/rs; y' = psum + xr --
            with tc.tile_pool(name="ps2", bufs=1, space="PSUM") as ps2:
                for j in range(NJ):
                    ofin = opool.tile([128, G, 512], BF16, tag="ofin", bufs=3,
                                      name=f"ofin{j}")
                    for cg in range(G):
                        o_ps = ps2.tile([128, 512], FP32, tag="o_ps", bufs=6,
                                        name=f"o_ps{j}_{cg}")
                        for e2 in range(G // 2):
                            nc.tensor.matmul(
                                o_ps[:],
                                at_f8[:, 2 * e2:2 * e2 + 2,
                                      cg * 128:(cg + 1) * 128],
                                xh_t[:, 2 * e2:2 * e2 + 2,
                                     j * 512:(j + 1) * 512],
                                start=(e2 == 0), stop=(e2 == G // 2 - 1),
                                perf_mode=DR)
                        if (j * G + cg) % 2 == 0:
                            # vector: direct PSUM + xr -> bf16
                            nc.vector.tensor_tensor(
                                ofin[:, cg, :], o_ps[:],
                                xr_t[:, cg, j * 512:(j + 1) * 512], op=ADD)
                        else:
                            # scalar copies PSUM out; gpsimd (no PSUM
                            # access) adds the residual SBUF-to-SBUF
                            ocp = opool.tile([128, 512], FP32, tag="ocp",
                                             bufs=3, name=f"ocp{j}_{cg}")
                            nc.scalar.copy(ocp[:], o_ps[:])
                            nc.gpsimd.tensor_tensor(
                                ofin[:, cg, :], ocp[:],
                                xr_t[:, cg, j * 512:(j + 1) * 512], op=ADD)
                    nc.sync.dma_start(y_v[:, :, j * 512:(j + 1) * 512],
                                      ofin[:])

    nc.compile()
    return nc


def _get_nc():
    if "nc" not in _CACHE:
        _CACHE["nc"] = _build_nc()
    return _CACHE["nc"]


def _make_in_maps(x, Wq, Wk, Wv, gamma):
    g = float(np.asarray(gamma).reshape(-1)[0])
    # scale 2^k chosen so the fp8 A^T entries sit near 2^9/rowsum * A
    k = int(round(9 - math.log2(max(abs(g), 2.0 ** -9))))
    k = max(4, min(k, 20))
    S = float(2.0 ** k)

    xb = np.ascontiguousarray(x.reshape(B, C, N).astype(np.float32))
    xh8 = xb.astype(ml_dtypes.float8_e4m3)
    xt8 = np.ascontiguousarray(xb.transpose(0, 2, 1)).astype(
        ml_dtypes.float8_e4m3)
    xt8 = np.ascontiguousarray(xt8)
    xr = (xb * S).astype(ml_dtypes.bfloat16)
    wqt = np.ascontiguousarray(Wq.T).astype(ml_dtypes.bfloat16)
    wkt = np.ascontiguousarray(Wk.T).astype(ml_dtypes.bfloat16)
    wvo = np.ascontiguousarray(Wv).astype(ml_dtypes.bfloat16)
    gcol = np.full((128, 1), g * S, np.float32)
    ocol = np.ones((128, 1), ml_dtypes.bfloat16)
    orow = np.ones((1, C), ml_dtypes.bfloat16)
    maps = []
    for i in range(B):
        maps.append({
            "xt": xt8[i], "xh": xh8[i], "xr": xr[i],
            "wqt": wqt, "wkt": wkt, "wvo": wvo,
            "gamma_col": gcol, "ones_col": ocol, "ones_row": orow,
        })
    return maps, S


def kernel(x, Wq, Wk, Wv, gamma, _trace=False, _trace_kwargs=None):
    nc = _get_nc()
    in_maps, S = _make_in_maps(np.asarray(x), np.asarray(Wq), np.asarray(Wk),
                               np.asarray(Wv), np.asarray(gamma))
    kwargs = {}
    if _trace:
        kwargs = dict(trace=True, **(_trace_kwargs or {}))
    res = bass_utils.run_bass_kernel_spmd(nc, in_maps,
                                          core_ids=list(range(B)), **kwargs)
    y = np.stack([res.results[i]["y"].astype(np.float32).reshape(C, HH, WW)
                  for i in range(B)])
    if _trace:
        kernel._last_result = res
    return (y * (1.0 / S)).astype(np.float32)


# revision 12
# speedup vs baseline: 1.5500x; 1.5500x over previous
"""Trainium2 Bass kernel for the contextual channel-attention transformer block.

Contract: kernel(**inputs) takes the FULL unsharded inputs
(x: (8,512,64,64) f32, Wq/Wk/Wv: (512,512) f32, gamma: (1,) f32) and
returns the FULL (8,512,64,64) f32 output.  Internally the batch is
data-parallel across 8 NeuronCores (one batch element per core).

All big GEMMs run as fp8e4 DoubleRow matmuls (2x PE rate).  Scales are
powers of two chosen so the softmax chain sees the same magnitudes as
the fp32 reference:
  xt/xh = fp8(x);  w* = fp8(8 W);  gx_f8 = 2^-6 Gx;  m3 = 2^-3 M3;
  G^T psum = 8 * 2^-3 * Wk M3q = G^T exactly;  msm_f8 = 2^8 Msm;
  A^T psum = 8 * 2^8 A^T;  f_row = S*gamma*2^-11 / rowsum;
  ph2 psum = S*(gamma*out) ;  y' = psum + S*x(bf16);  host: y = y'/S.

Host pre-swizzles every tensor to a [128, *] partition-major layout so
each DMA is 128 contiguous lines (cheap descriptors, full bandwidth).
"""

import os
import sys

for _p in ("/opt/trn_rl_repo", "/root/.axon_site/_ro/trn_rl_repo"):
    if os.path.isdir(_p) and _p not in sys.path:
        sys.path.insert(0, _p)

import math

import ml_dtypes
import numpy as np

import concourse.bass as bass
import concourse.tile as tile
from concourse import bacc, bass_utils, mybir

# Problem constants (hardcoded; kernel.py must be self-contained).
B, C, HH, WW = 8, 512, 64, 64
N = HH * WW          # 4096 spatial positions
G = C // 128         # 4 channel groups of 128
N1 = N // 128        # 32 Gram chunks (128 spatial each)
NJ = N // 512        # 8 output chunks (512 spatial each)
XT_PIECES = (2, 2, 4, 8, 16)   # progressive xt chunks (units of 128 spatial)
EPS = 1e-6
INV_H = 4.0          # 1 / 0.25 temperature
FP32 = mybir.dt.float32
BF16 = mybir.dt.bfloat16
FP8 = mybir.dt.float8e4
DR = mybir.MatmulPerfMode.DoubleRow

_CACHE = {}


def _build_nc():
    nc = bacc.Bacc("TRN2", target_bir_lowering=False)

    xt_d = nc.dram_tensor("xt", [128, N1 * C], FP8, kind="ExternalInput")
    xh_d = nc.dram_tensor("xh", [128, G * N], FP8, kind="ExternalInput")
    xr_d = nc.dram_tensor("xr", [128, G * N], BF16, kind="ExternalInput")
    wq_d = nc.dram_tensor("wq8", [128, G * C], FP8, kind="ExternalInput")
    wk_d = nc.dram_tensor("wk8", [128, G * C], FP8, kind="ExternalInput")
    wv_d = nc.dram_tensor("wv8", [128, G * C], FP8, kind="ExternalInput")
    gcol_d = nc.dram_tensor("gamma_col", [1, 1], FP32, kind="ExternalInput")
    ecol_d = nc.dram_tensor("eps_col", [1, 1], FP32, kind="ExternalInput")
    ocol_d = nc.dram_tensor("ones_col", [128, 1], BF16, kind="ExternalInput")
    ocol8_d = nc.dram_tensor("ones_col8", [128, 1], FP8, kind="ExternalInput")
    orow_d = nc.dram_tensor("ones_row", [1, 128], BF16, kind="ExternalInput")
    y_d = nc.dram_tensor("y", [128, NJ * G * 512], BF16,
                         kind="ExternalOutput")

    xt_v = xt_d.ap().rearrange("p (i c) -> p i c", c=C)      # [128, N1, C]
    xh_v = xh_d.ap().rearrange("p (g n) -> p g n", n=N)      # [128, G, N]
    xr_v = xr_d.ap().rearrange("p (g n) -> p g n", n=N)
    wq_v = wq_d.ap().rearrange("p (g o) -> p g o", o=C)      # [128, G, C]
    wk_v = wk_d.ap().rearrange("p (g o) -> p g o", o=C)
    wv_v = wv_d.ap().rearrange("p (g o) -> p g o", o=C)
    y_v = y_d.ap().rearrange("p (j g n) -> p j g n", g=G, n=512)

    MUL = mybir.AluOpType.mult
    ADD = mybir.AluOpType.add
    MIN = mybir.AluOpType.min
    AX = mybir.AxisListType.X
    Exp = mybir.ActivationFunctionType.Exp
    Ln = mybir.ActivationFunctionType.Ln
    Copy = mybir.ActivationFunctionType.Copy

    with tile.TileContext(nc) as tc:
        with (
            tc.tile_pool(name="consts", bufs=1) as cpool,
            tc.tile_pool(name="weights", bufs=1) as wpool,
            tc.tile_pool(name="xt", bufs=1) as xtpool,
            tc.tile_pool(name="xbig", bufs=1) as xbpool,
            tc.tile_pool(name="gram", bufs=1) as gpool,
            tc.tile_pool(name="small", bufs=2) as spool,
            tc.tile_pool(name="mid", bufs=3) as mpool,
            tc.tile_pool(name="msm", bufs=1) as msmpool,
            tc.tile_pool(name="outs", bufs=4) as opool,
        ):
            # ---- input DMAs, all issued on sync in priority order --------
            xt_t = []
            off = 0
            for pi, sz in enumerate(XT_PIECES):
                t = xtpool.tile([128, sz, C], FP8, tag=f"xt{pi}", bufs=1,
                                name=f"xt{pi}")
                nc.sync.dma_start(t[:], xt_v[:, off:off + sz, :])
                xt_t.append((off, sz, t))
                off += sz

            ones_col = cpool.tile([128, 1], BF16, tag="ones_col")
            nc.scalar.dma_start(ones_col[:], ocol_d.ap())
            ones_col8 = cpool.tile([128, 1], FP8, tag="ones_col8")
            nc.scalar.dma_start(ones_col8[:], ocol8_d.ap())
            ones_row = cpool.tile([1, 128], BF16, tag="ones_row")
            nc.scalar.dma_start(ones_row[:], orow_d.ap())
            gamma_col = cpool.tile([1, 1], FP32, tag="gamma_col")
            nc.scalar.dma_start(gamma_col[:], gcol_d.ap())
            eps_col = cpool.tile([1, 1], FP32, tag="eps_col")
            nc.scalar.dma_start(eps_col[:], ecol_d.ap())

            wq = wpool.tile([128, G, C], FP8, tag="wq")
            wk = wpool.tile([128, G, C], FP8, tag="wk")
            wv = wpool.tile([128, G, C], FP8, tag="wv")
            nc.sync.dma_start(wq[:], wq_v)
            nc.sync.dma_start(wk[:], wk_v)
            nc.sync.dma_start(wv[:], wv_v)

            xh_t = xbpool.tile([128, G, N], FP8, tag="xh", name="xh")
            nc.sync.dma_start(xh_t[:], xh_v)
            xr_t = xbpool.tile([128, G, N], BF16, tag="xr", name="xr")
            nc.sync.dma_start(xr_t[:], xr_v)

            # ---- Gx = X X^T  (fp8 DoubleRow, accumulated over 16 pairs) --
            gx_f8 = gpool.tile([128, G, C], FP8, tag="gx_f8")
            with tc.tile_pool(name="psGx", bufs=1, space="PSUM") as psGx:
                gx_ps = [psGx.tile([128, C], FP32, tag="gx", bufs=G,
                                   name=f"gx{cg}") for cg in range(G)]
                first = True
                for off, sz, t in xt_t:
                    for lp in range(sz // 2):
                        last = (off + 2 * lp + 2 == N1)
                        for cg in range(G):
                            nc.tensor.matmul(
                                gx_ps[cg][:],
                                t[:, 2 * lp:2 * lp + 2,
                                  cg * 128:(cg + 1) * 128],
                                t[:, 2 * lp:2 * lp + 2, :],
                                start=first, stop=last, perf_mode=DR)
                        first = False
                for cg in range(G):
                    if cg % 2:
                        nc.scalar.activation(gx_f8[:, cg, :], gx_ps[cg][:],
                                             Copy, scale=2.0 ** -6)
                    else:
                        nc.vector.tensor_scalar(gx_f8[:, cg, :], gx_ps[cg][:],
                                                2.0 ** -6, None, op0=MUL)

            # ---- M3q = 2^-3 Gx Wq^T, M3k = 2^-3 Gx Wk^T (fp8 DR) ---------
            m3q = gpool.tile([128, G, C], FP8, tag="m3q")
            m3k = gpool.tile([128, G, C], FP8, tag="m3k")
            with tc.tile_pool(name="psM3", bufs=1, space="PSUM") as psM3:
                for cg in range(G):
                    q_ps = psM3.tile([128, C], FP32, tag="m3q", bufs=G,
                                     name=f"m3q{cg}")
                    k_ps = psM3.tile([128, C], FP32, tag="m3k", bufs=G,
                                     name=f"m3k{cg}")
                    for g2 in range(G // 2):
                        lhs = gx_f8[:, 2 * g2:2 * g2 + 2,
                                    cg * 128:(cg + 1) * 128]
                        nc.tensor.matmul(q_ps[:], lhs,
                                         wq[:, 2 * g2:2 * g2 + 2, :],
                                         start=(g2 == 0),
                                         stop=(g2 == G // 2 - 1),
                                         perf_mode=DR)
                        nc.tensor.matmul(k_ps[:], lhs,
                                         wk[:, 2 * g2:2 * g2 + 2, :],
                                         start=(g2 == 0),
                                         stop=(g2 == G // 2 - 1),
                                         perf_mode=DR)
                    nc.scalar.copy(m3q[:, cg, :], q_ps[:])
                    nc.vector.tensor_copy(m3k[:, cg, :], k_ps[:])

            msm = msmpool.tile([128, G, C], FP8, tag="msm")
            at_f8 = gpool.tile([128, G, C], FP8, tag="at_f8")
            with tc.tile_pool(name="psN", bufs=1, space="PSUM") as psN:
                # ---- norms: |Q_c|^2 row, |K_d|^2 columns -----------------
                # tq = (8Wq)*(2^-3 M3q) = Wq o M3q exactly
                sqq = psN.tile([1, C], FP32, tag="sqq", name="sqq")
                sqk_ps = [psN.tile([128, 1], FP32, tag="sqk", bufs=G,
                                   name=f"sqk{d}") for d in range(G)]
                for g in range(G):
                    tq = mpool.tile([128, C], BF16, tag="tq")
                    nc.vector.tensor_tensor(tq[:], wq[:, g, :], m3q[:, g, :],
                                            op=MUL)
                    nc.tensor.matmul(sqq[:], ones_col[:], tq[:],
                                     start=(g == 0), stop=(g == G - 1))
                    tk = mpool.tile([128, C], BF16, tag="tk")
                    nc.gpsimd.tensor_tensor(tk[:], wk[:, g, :],
                                            m3k[:, g, :], op=MUL)
                    for dg in range(G):
                        nc.tensor.matmul(sqk_ps[dg][:],
                                         tk[:, dg * 128:(dg + 1) * 128],
                                         ones_col[:],
                                         start=(g == 0), stop=(g == G - 1))

                # 1/sqrt(s) = exp(-0.5*ln(s))
                ln_q = spool.tile([1, C], FP32, tag="ln_q")
                nc.scalar.activation(ln_q[:], sqq[:], Ln)
                ln_ks = []
                for dg in range(G):
                    ln_k = spool.tile([128, 1], FP32, tag="ln_k", bufs=G,
                                      name=f"ln_k{dg}")
                    nc.scalar.activation(ln_k[:], sqk_ps[dg][:], Ln)
                    ln_ks.append(ln_k)
                rq_bf = spool.tile([1, C], BF16, tag="rq_bf")
                nc.scalar.activation(rq_bf[:], ln_q[:], Exp, scale=-0.5)
                rk_cols = []
                for dg in range(G):
                    rk = spool.tile([128, 1], FP32, tag="rk", bufs=G,
                                    name=f"rk{dg}")
                    nc.scalar.activation(rk[:], ln_ks[dg][:], Exp, scale=-0.5)
                    rk_cols.append(rk)

                bq_ps = psN.tile([128, C], FP32, tag="bq_ps", name="bq_ps")
                nc.tensor.matmul(bq_ps[:], ones_row[:], rq_bf[:],
                                 start=True, stop=True)
                bq = mpool.tile([128, C], FP32, tag="bq", bufs=1)
                nc.scalar.copy(bq[:], bq_ps[:])

            with tc.tile_pool(name="psB", bufs=1, space="PSUM") as psB:
                # ---- G^T per d-group + softmax + A^T ---------------------
                # G^T psum = (8Wk)(2^-3 M3q) = G^T exactly.
                # cos = G^T * rq_c * rk_d; rk folds into the Exp scale and
                # the min-reduce (rk > 0), so no [128,C] cos tile is built.
                at_ps = [psB.tile([128, C], FP32, tag="at", bufs=G,
                                  name=f"at{eg}") for eg in range(G)]
                s_row = psB.tile([1, C], FP32, tag="s_row", name="s_row")
                for dg in range(G):
                    g_ps = psB.tile([128, C], FP32, tag="g_ps", bufs=2,
                                    name=f"g_ps{dg}")
                    for g2 in range(G // 2):
                        nc.tensor.matmul(g_ps[:],
                                         wk[:, 2 * g2:2 * g2 + 2,
                                            dg * 128:(dg + 1) * 128],
                                         m3q[:, 2 * g2:2 * g2 + 2, :],
                                         start=(g2 == 0),
                                         stop=(g2 == G // 2 - 1),
                                         perf_mode=DR)
                    t1 = mpool.tile([128, C], FP32, tag="t1")
                    nc.vector.tensor_tensor(t1[:], g_ps[:], bq[:], op=MUL)
                    mn0 = spool.tile([128, 1], FP32, tag="mn0")
                    nc.vector.tensor_reduce(mn0[:], t1[:], axis=AX, op=MIN)
                    mn = spool.tile([128, 1], FP32, tag="mn")
                    nc.vector.tensor_tensor(mn[:], mn0[:], rk_cols[dg][:],
                                            op=MUL)
                    den = spool.tile([128, 1], FP32, tag="den")
                    nc.vector.tensor_scalar(den[:], mn[:], -1.0, 1.0 + EPS,
                                            op0=MUL, op1=ADD)
                    r = spool.tile([128, 1], FP32, tag="r")
                    nc.vector.reciprocal(r[:], den[:])
                    r4 = spool.tile([128, 1], FP32, tag="r4")
                    nc.vector.tensor_scalar(r4[:], r[:], INV_H, 0.0,
                                            op0=MUL, op1=ADD)
                    svk = spool.tile([128, 1], FP32, tag="svk")
                    nc.vector.tensor_tensor(svk[:], r4[:], rk_cols[dg][:],
                                            op=MUL)
                    bv = spool.tile([128, 1], FP32, tag="bv")
                    nc.vector.tensor_scalar(bv[:], r[:], -INV_H, 1.0,
                                            op0=MUL, op1=ADD)
                    e = mpool.tile([128, C], BF16, tag="e")
                    se = spool.tile([128, 1], FP32, tag="se")
                    nc.scalar.activation(e[:], t1[:], Exp,
                                         bias=bv[:], scale=svk[:],
                                         accum_out=se[:])
                    rd = spool.tile([128, 1], FP32, tag="rd")
                    nc.vector.reciprocal(rd[:], se[:])
                    nc.vector.tensor_scalar(msm[:, dg, :], e[:], rd[:],
                                            256.0, op0=MUL, op1=MUL)
                    # A^T accumulation over d (pairs) + rowsum row
                    nc.tensor.matmul(s_row[:], ones_col8[:], msm[:, dg, :],
                                     start=(dg == 0), stop=(dg == G - 1))
                    if dg % 2 == 1:
                        for eg in range(G):
                            nc.tensor.matmul(
                                at_ps[eg][:],
                                wv[:, dg - 1:dg + 1,
                                   eg * 128:(eg + 1) * 128],
                                msm[:, dg - 1:dg + 1, :],
                                start=(dg == 1), stop=(dg == G - 1),
                                perf_mode=DR)

                # ---- f_row = S*gamma*2^-11 / (rowsum+eps), on scalar -----
                lns = spool.tile([1, C], FP32, tag="lns", bufs=1)
                nc.scalar.activation(lns[:], s_row[:], Ln,
                                     scale=2.0 ** -8, bias=eps_col[:])
                rec = spool.tile([1, C], FP32, tag="rec", bufs=1)
                nc.scalar.activation(rec[:], lns[:], Exp, scale=-1.0)
                f_row = spool.tile([1, C], BF16, tag="f_row", bufs=1)
                nc.scalar.activation(f_row[:], rec[:], Copy,
                                     scale=gamma_col[:])
                fbc_ps = psB.tile([128, C], FP32, tag="fbc_ps", name="fbc_ps")
                nc.tensor.matmul(fbc_ps[:], ones_row[:], f_row[:],
                                 start=True, stop=True)
                fbc = mpool.tile([128, C], FP32, tag="fbc", bufs=1)
                nc.scalar.copy(fbc[:], fbc_ps[:])

                # ---- A^T * f -> fp8 (PSUM reads -> vector) ---------------
                for eg in range(G):
                    nc.vector.tensor_tensor(at_f8[:, eg, :], at_ps[eg][:],
                                            fbc[:], op=MUL)

            # ---- phase 2: psum = S*gamma*out; y' = psum + S*x ------------
            with tc.tile_pool(name="ps2", bufs=1, space="PSUM") as ps2:
                for j in range(NJ):
                    ofin = opool.tile([128, G, 512], BF16, tag="ofin", bufs=3,
                                      name=f"ofin{j}")
                    for cg in range(G):
                        o_ps = ps2.tile([128, 512], FP32, tag="o_ps", bufs=6,
                                        name=f"o_ps{j}_{cg}")
                        for e2 in range(G // 2):
                            nc.tensor.matmul(
                                o_ps[:],
                                at_f8[:, 2 * e2:2 * e2 + 2,
                                      cg * 128:(cg + 1) * 128],
                                xh_t[:, 2 * e2:2 * e2 + 2,
                                     j * 512:(j + 1) * 512],
                                start=(e2 == 0), stop=(e2 == G // 2 - 1),
                                perf_mode=DR)
                        m = j * G + cg
                        if m % 8 < 5:
                            # vector: direct PSUM + xr -> bf16
                            nc.vector.tensor_tensor(
                                ofin[:, cg, :], o_ps[:],
                                xr_t[:, cg, j * 512:(j + 1) * 512], op=ADD)
                        else:
                            # scalar copies PSUM out; gpsimd adds SBUF-SBUF
                            ocp = opool.tile([128, 512], BF16, tag="ocp",
                                             bufs=3, name=f"ocp{j}_{cg}")
                            nc.scalar.copy(ocp[:], o_ps[:])
                            nc.gpsimd.tensor_tensor(
                                ofin[:, cg, :], ocp[:],
                                xr_t[:, cg, j * 512:(j + 1) * 512], op=ADD)
                    nc.sync.dma_start(y_v[:, j, :, :], ofin[:])

    nc.compile()
    return nc


def _get_nc():
    if "nc" not in _CACHE:
        _CACHE["nc"] = _build_nc()
    return _CACHE["nc"]


def _swz(a, blocks):
    """[blocks*128, M] -> [128, blocks*M] partition-major."""
    b2, m = a.shape
    return np.ascontiguousarray(
        a.reshape(blocks, 128, m).transpose(1, 0, 2).reshape(128, blocks * m))


def _make_in_maps(x, Wq, Wk, Wv, gamma):
    g = float(np.asarray(gamma).reshape(-1)[0])
    # scale 2^k chosen so the fp8 A^T entries sit near 2^9/rowsum * A
    k = int(round(9 - math.log2(max(abs(g), 2.0 ** -9))))
    k = max(4, min(k, 20))
    S = float(2.0 ** k)

    f8 = ml_dtypes.float8_e4m3
    bf = ml_dtypes.bfloat16
    xb = np.ascontiguousarray(x.reshape(B, C, N).astype(np.float32))
    xh8 = np.stack([_swz(xb[i].astype(f8), G) for i in range(B)])
    xt8 = np.stack([_swz(np.ascontiguousarray(xb[i].T).astype(f8), N1)
                    for i in range(B)])
    xr = np.stack([_swz((xb[i] * S).astype(bf), G) for i in range(B)])
    wq8 = _swz(np.ascontiguousarray(Wq.T * 8.0).astype(f8), G)
    wk8 = _swz(np.ascontiguousarray(Wk.T * 8.0).astype(f8), G)
    wv8 = _swz(np.ascontiguousarray(Wv * 8.0).astype(f8), G)
    gcol = np.full((1, 1), g * S * 2.0 ** -11, np.float32)
    ecol = np.full((1, 1), EPS, np.float32)
    ocol = np.ones((128, 1), bf)
    ocol8 = np.ones((128, 1), f8)
    orow = np.ones((1, 128), bf)
    maps = []
    for i in range(B):
        maps.append({
            "xt": xt8[i], "xh": xh8[i], "xr": xr[i],
            "wq8": wq8, "wk8": wk8, "wv8": wv8,
            "gamma_col": gcol, "eps_col": ecol,
            "ones_col": ocol, "ones_col8": ocol8,
            "ones_row": orow,
        })
    return maps, S


def kernel(x, Wq, Wk, Wv, gamma, _trace=False, _trace_kwargs=None):
    nc = _get_nc()
    in_maps, S = _make_in_maps(np.asarray(x), np.asarray(Wq), np.asarray(Wk),
                               np.asarray(Wv), np.asarray(gamma))
    kwargs = {}
    if _trace:
        kwargs = dict(trace=True, **(_trace_kwargs or {}))
    res = bass_utils.run_bass_kernel_spmd(nc, in_maps,
                                          core_ids=list(range(B)), **kwargs)
    # y DRAM layout: [128, NJ, G, 512] -> (C=(g,p), N=(j,n'))
    y = np.stack([
        res.results[i]["y"].astype(np.float32)
        .reshape(128, NJ, G, 512).transpose(2, 0, 1, 3).reshape(C, HH, WW)
        for i in range(B)])
    if _trace:
        kernel._last_result = res
    return (y * (1.0 / S)).astype(np.float32)
